# revision 4
# baseline (speedup 1.0000x reference)
"""Batched weighted least squares (BasicLS) on 8 Trainium2 NeuronCores.

For each batch b: A = [-x[:,1:4], 1], r = x[:,0]; y = pinv(A) @ r.
A is 32x4 Gaussian => full rank => y = (A^T A)^-1 A^T r.  All required
statistics are entries of the 5x5 augmented Gram of [r, x1, x2, x3, 1]:
13 per-batch scalars (4 sums + 9 pairwise product-sums over m).

Sign substitution: with J = diag(-1,-1,-1,1), G = J K J where
K = [[S11,S12,S13,T1],[S12,S22,S23,T2],[S13,S23,S33,T3],[T1,T2,T3,32]]
(all-positive form), and G y = rhs becomes K z = c with
c = [S01,S02,S03,T0], y = [-z0,-z1,-z2,z3].

K z = c is solved per batch with Cramer's rule via shared 2x2 minors
(rows (0,1) and rows (2,3)) over the 5 columns [K | c]: 20 minors, five
4x4 determinants, one reciprocal.  All solve ops are elementwise on
[128, 256] fp32 tiles (one entry per batch).

Sharding: pure data parallel over the batch dim, 32768 batches per core.
"""

import itertools

import numpy as np

import concourse.bacc as bacc
import concourse.tile as tile
from concourse import mybir
from concourse.bass_utils import run_bass_kernel_spmd

F32 = mybir.dt.float32
AX = mybir.AxisListType

B, M, D = 262144, 32, 4
NCORES = 8
BC = B // NCORES          # 32768 batches per core
NT = 8                    # DMA tiles per core
TB = BC // NT             # 4096 batches per tile
CPT = TB // 128           # 32 batches per partition per tile
W = NT * CPT              # 256 stat columns per partition

# stat indices in the [128, 13, W] stats tile
iT0, iT1, iT2, iT3 = 0, 1, 2, 3
iS01, iS02, iS03 = 4, 5, 6
iS11, iS12, iS13, iS22, iS23, iS33 = 7, 8, 9, 10, 11, 12


def _emit(nc, tc, xd, yd):
    V, G, A = nc.vector, nc.gpsimd, nc.scalar

    # batch b = t*TB + p*CPT + c  (tile, partition, col)
    x_all = xd.ap().rearrange("(t p c) m d -> t p c m d", t=NT, p=128)
    y_all = yd.ap().rearrange("(t p c) d -> p t c d", t=NT, p=128)

    with (
        tc.tile_pool(name="xin", bufs=2) as xpool,
        tc.tile_pool(name="xp", bufs=2) as xppool,
        tc.tile_pool(name="prod", bufs=6) as ppool,
        tc.tile_pool(name="stat", bufs=1) as spool,
        tc.tile_pool(name="solve", bufs=1) as lpool,
        tc.tile_pool(name="pp", bufs=6) as pppool,
        tc.tile_pool(name="acc", bufs=4) as apool,
    ):
        ST = spool.tile([128, 13, W], F32)
        CROSS = [(iS01, 0, 1), (iS02, 0, 2), (iS03, 0, 3),
                 (iS12, 1, 2), (iS13, 1, 3), (iS23, 2, 3)]
        SQ = [(iS11, 1), (iS22, 2), (iS33, 3)]

        for t in range(NT):
            cs, ce = t * CPT, (t + 1) * CPT
            xt = xpool.tile([128, CPT, M, D], F32, tag="xt")
            nc.sync.dma_start(out=xt, in_=x_all[t])
            # repack (c, m, d) -> (d, c, m) so every downstream op is
            # unit-stride (ACT engine, otherwise idle in the stats phase)
            xp = xppool.tile([128, D, CPT, M], F32, tag="xp")
            A.copy(out=xp, in_=xt.rearrange("p c m d -> p d c m"))
            # T_d sums for all 4 features in one reduce
            V.reduce_sum(out=ST[:, 0:4, cs:ce], in_=xp, axis=AX.X)
            for (s, j, k) in CROSS:
                pt = ppool.tile([128, CPT, M], F32, tag="prod")
                G.tensor_mul(out=pt, in0=xp[:, j], in1=xp[:, k])
                V.reduce_sum(out=ST[:, s, cs:ce], in_=pt, axis=AX.X)
            for (s, j) in SQ:
                pt = ppool.tile([128, CPT, M], F32, tag="prod")
                A.square(out=pt, in_=xp[:, j])
                V.reduce_sum(out=ST[:, s, cs:ce], in_=pt, axis=AX.X)

        # ---------------- solve ----------------
        def stat(s):
            return ST[:, s, :]

        a, b, c_, d = stat(iS11), stat(iS12), stat(iS13), stat(iT1)
        e, f_, g = stat(iS22), stat(iS23), stat(iT2)
        h, i_ = stat(iS33), stat(iT3)
        r0, r1, r2, r3 = stat(iS01), stat(iS02), stat(iS03), stat(iT0)

        sched = itertools.cycle([V, V, V, V, G, G, G])

        def tmp(name, pool=None, tag=None):
            pool = pool or lpool
            return pool.tile([128, W], F32, tag=tag or name, name=name)

        def emul(u, v, name, pool=None, tag=None):
            t_ = tmp(name, pool, tag)
            next(sched).tensor_mul(out=t_, in0=u, in1=v)
            return t_

        def esub(u, v, name, pool=None, tag=None):
            t_ = tmp(name, pool, tag)
            next(sched).tensor_sub(out=t_, in0=u, in1=v)
            return t_

        def eadd(u, v, name, pool=None, tag=None):
            t_ = tmp(name, pool, tag)
            next(sched).tensor_add(out=t_, in0=u, in1=v)
            return t_

        def m2(u, v, w, x_, name):  # u*v - w*x
            p1 = emul(u, v, name + "p1", pppool, "pp")
            p2 = emul(w, x_, name + "p2", pppool, "pp")
            return esub(p1, p2, name)

        def asq(u, name):  # u*u on the scalar engine
            t_ = tmp(name)
            A.square(out=t_, in_=u)
            return t_

        def amul32(u, name):  # 32*u on the scalar engine
            t_ = tmp(name)
            A.mul(out=t_, in_=u, mul=32.0)
            return t_

        # 2x2 minors over columns [0..3, R]; rows (0,1) = s, rows (2,3) = t
        b2 = asq(b, "b2")
        i2 = asq(i_, "i2")
        ae = emul(a, e, "aep1", pppool, "pp")
        s01 = esub(ae, b2, "s01")
        s02 = m2(a, f_, c_, b, "s02")
        s03 = m2(a, g, d, b, "s03")
        s12 = m2(b, f_, c_, e, "s12")
        s13 = m2(b, g, d, e, "s13")
        s23 = m2(c_, g, d, f_, "s23")
        s0R = m2(a, r1, r0, b, "s0R")
        s1R = m2(b, r1, r0, e, "s1R")
        s2R = m2(c_, r1, r0, f_, "s2R")
        s3R = m2(d, r1, r0, g, "s3R")
        t01 = s23  # identical expression: c*g - d*f
        t02 = m2(c_, i_, h, d, "t02")
        t12 = m2(f_, i_, h, g, "t12")
        c32 = amul32(c_, "c32")
        id_ = emul(i_, d, "idp1", pppool, "pp")
        t03 = esub(c32, id_, "t03")
        fq32 = amul32(f_, "fq32")
        ig = emul(i_, g, "igp1", pppool, "pp")
        t13 = esub(fq32, ig, "t13")
        h32 = amul32(h, "h32")
        t23 = esub(h32, i2, "t23")
        t0R = m2(c_, r3, r2, d, "t0R")
        t1R = m2(f_, r3, r2, g, "t1R")
        t2R = m2(h, r3, r2, i_, "t2R")
        ir3 = emul(i_, r3, "ir3p1", pppool, "pp")
        r232 = amul32(r2, "r232")
        t3R = esub(ir3, r232, "t3R")

        Smap = {(0, 1): s01, (0, 2): s02, (0, 3): s03, (1, 2): s12,
                (1, 3): s13, (2, 3): s23, (0, 4): s0R, (1, 4): s1R,
                (2, 4): s2R, (3, 4): s3R}
        Tmap = {(0, 1): t01, (0, 2): t02, (0, 3): t03, (1, 2): t12,
                (1, 3): t13, (2, 3): t23, (0, 4): t0R, (1, 4): t1R,
                (2, 4): t2R, (3, 4): t3R}

        def det_terms(cols):
            # Laplace expansion of the 4x4 det over row pairs (0,1)/(2,3)
            terms = []
            for (p, q, sgn) in [(0, 1, 1.0), (0, 2, -1.0), (0, 3, 1.0),
                                (1, 2, 1.0), (1, 3, -1.0), (2, 3, 1.0)]:
                rest = [cols[k] for k in range(4) if k not in (p, q)]
                cp, cq = cols[p], cols[q]
                ss = 1.0
                if cp > cq:
                    cp, cq, ss = cq, cp, -1.0
                ta, tb = rest
                ts = 1.0
                if ta > tb:
                    ta, tb, ts = tb, ta, -1.0
                terms.append((sgn * ss * ts, Smap[(cp, cq)], Tmap[(ta, tb)]))
            terms.sort(key=lambda z: -z[0])  # a positive term first
            return terms

        def emit_det(cols, name):
            acc = None
            for idx, (sgn, sv, tv) in enumerate(det_terms(cols)):
                if acc is None:
                    assert sgn > 0
                    acc = emul(sv, tv, f"{name}m{idx}", apool, "acc")
                else:
                    prod = emul(sv, tv, f"{name}m{idx}", pppool, "pp")
                    if sgn > 0:
                        acc = eadd(acc, prod, f"{name}a{idx}", apool, "acc")
                    else:
                        acc = esub(acc, prod, f"{name}a{idx}", apool, "acc")
            return acc

        det = emit_det([0, 1, 2, 3], "dt")
        dR0 = emit_det([4, 1, 2, 3], "d0")
        dR1 = emit_det([0, 4, 2, 3], "d1")
        dR2 = emit_det([0, 1, 4, 3], "d2")
        dR3 = emit_det([0, 1, 2, 4], "d3")

        rdet = tmp("rdet")
        scratch = tmp("rscratch")
        V.reciprocal_approx_accurate(out=rdet, in_=det, scratch=scratch)
        nrdet = tmp("nrdet")
        V.tensor_scalar_mul(out=nrdet, in0=rdet, scalar1=-1.0)

        OUT = lpool.tile([128, NT, CPT, D], F32, tag="OUT", name="OUT")
        for comp, (dv, rv) in enumerate(
            [(dR0, nrdet), (dR1, nrdet), (dR2, nrdet), (dR3, rdet)]
        ):
            ov = OUT[:, :, :, comp]                      # [128, NT, CPT]
            dv3 = dv.rearrange("p (t c) -> p t c", t=NT)
            rv3 = rv.rearrange("p (t c) -> p t c", t=NT)
            next(sched).tensor_mul(out=ov, in0=dv3, in1=rv3)
        nc.sync.dma_start(out=y_all, in_=OUT)


_NC_CACHE = {}


def _get_nc():
    if "nc" not in _NC_CACHE:
        nc = bacc.Bacc("TRN2", target_bir_lowering=False, debug=False,
                       num_devices=NCORES)
        xd = nc.dram_tensor("x", [BC, M, D], F32, kind="ExternalInput")
        yd = nc.dram_tensor("y", [BC, D], F32, kind="ExternalOutput")
        with tile.TileContext(nc) as tc:
            _emit(nc, tc, xd, yd)
        nc.compile()
        _NC_CACHE["nc"] = nc
    return _NC_CACHE["nc"]


def run_sharded(x, trace=False, **kwargs):
    """x: [B, 32, 4] fp32 -> (out [B, 4] fp32, BassKernelResults)."""
    nc = _get_nc()
    in_maps = [
        {"x": np.ascontiguousarray(x[k * BC:(k + 1) * BC])}
        for k in range(NCORES)
    ]
    res = run_bass_kernel_spmd(nc, in_maps, core_ids=list(range(NCORES)),
                               trace=trace, **kwargs)
    out = np.concatenate([res.results[k]["y"] for k in range(NCORES)], axis=0)
    return out, res


def kernel(**inputs):
    x = np.asarray(inputs["x"], dtype=np.float32)
    out, _ = run_sharded(x)
    return out


# revision 5
# speedup vs baseline: 21386.5437x; 21386.5437x over previous
"""BasicLS on 8 trn2 cores — strategy B: fp16 + TensorEngine reductions.

Pipeline per 4096-batch tile:
  1. DMA x tile [128, (c,m,d)] fp32.
  2. ACT cast+swizzle -> xh [128, (d, q, g, m)] fp16   (c = 4q+g)
  3. PE transposes of the 32 [128,(g,m)] blocks -> PSUM -> DVE copy ->
     Fall [128=(g,m), (d, q, p)] fp16  (feature-major, m on partitions)
  4. DVE/GPS fp16 products for the 9 pairwise stats.
  5. PE matmuls with a sliding ones-pattern weight reduce over m into
     PSUM stats [52=(4s+g), 512]; fp32 accumulation.
  6. ACT copy PSUM->SBUF; PE transposes stat chunks back to
     ST2 [128=p, t, q, 52=(4s+g)] so each batch's 13 stats live in its
     own partition's free dim.
  7. Cramer solve (shared 2x2 minors) on [128, 8, 8, 4] views; output
     assembled and DMA'd as [BC, 4].
"""

import itertools

import numpy as np

import concourse.bacc as bacc
import concourse.tile as tile
from concourse import mybir
from concourse.bass_utils import run_bass_kernel_spmd
from concourse.masks import make_identity

F32 = mybir.dt.float32
F16 = mybir.dt.float16

B, M, D = 262144, 32, 4
NCORES = 8
BC = B // NCORES          # 32768
NT = 8
TB = BC // NT             # 4096
CPT = TB // 128           # 32 (c = 4q + g, q:8, g:4)
NQ, NG = 8, 4
W = NT * CPT              # 256

# stat order: 0..3 = T0..T3; 4 S01, 5 S02, 6 S03, 7 S11, 8 S12, 9 S13,
# 10 S22, 11 S23, 12 S33
CROSS = [(4, 0, 1), (5, 0, 2), (6, 0, 3), (8, 1, 2), (9, 1, 3), (11, 2, 3)]
SQ = [(7, 1), (10, 2), (12, 3)]
NS = 13


def _emit(nc, tc, xd, yd):
    V, G, A = nc.vector, nc.gpsimd, nc.scalar

    x_all = xd.ap().rearrange("(t p c) m d -> t p c m d", t=NT, p=128)
    y_all = yd.ap().rearrange("(t p c) d -> p t c d", t=NT, p=128)

    with (
        tc.tile_pool(name="const", bufs=1) as cpool,
        tc.tile_pool(name="xin", bufs=3) as xpool,
        tc.tile_pool(name="xh", bufs=2) as xhpool,
        tc.tile_pool(name="fall", bufs=3) as fpool,
        tc.tile_pool(name="pr", bufs=12) as prpool,
        tc.tile_pool(name="sst", bufs=3) as sspool,
        tc.tile_pool(name="stat", bufs=1) as spool,
        tc.tile_pool(name="solve", bufs=1) as lpool,
        tc.tile_pool(name="pp", bufs=6) as pppool,
        tc.tile_pool(name="acc", bufs=4) as apool,
        tc.tile_pool(name="pst", bufs=2, space="PSUM") as ptpool,
        tc.tile_pool(name="psp", bufs=2, space="PSUM") as sppool,
        tc.tile_pool(name="ps2", bufs=2, space="PSUM") as p2pool,
    ):
        ident16 = cpool.tile([128, 128], F16, name="ident16")
        make_identity(nc, ident16)
        ident32 = cpool.tile([128, 128], F32, name="ident32")
        make_identity(nc, ident32)
        # master ones-pattern weight: MW[32g+m, 48+g] = 1.
        # For stat s, lhsT = MW[:, 48-4s : 100-4s] places the group-g m-sum
        # of the rhs at output partition 4s+g.
        MW = cpool.tile([128, 100], F16, name="MW")
        V.memset(MW, 0.0)
        for g in range(NG):
            V.memset(MW[32 * g:32 * (g + 1), 48 + g:49 + g], 1.0)

        import os as _os
        _skip_stats = _os.environ.get("KB_SKIP_STATS") == "1"
        _skip_solve = _os.environ.get("KB_SKIP_SOLVE") == "1"
        HT = NT // 2  # tiles per solve half

        # per-batch stats, physically split by t-half so the first solve
        # half's dependencies close after tile 3
        ST2h = [
            spool.tile([128, NT // 2, NQ, 52], F32, name=f"ST2_{hh}",
                       tag=f"ST2_{hh}")
            for hh in range(2)
        ]
        if _skip_stats:
            V.memset(ST2h[0], 1.0)
            V.memset(ST2h[1], 1.0)

        OUT = lpool.tile([128, NT, CPT, D], F32, tag="OUT", name="OUT")
        OUT5 = OUT.rearrange("p t (q g) d -> p t q g d", q=NQ)

        def emit_solve(hh):
            def stat(s):
                return ST2h[hh][:, :, :, 4 * s:4 * s + 4]

            a, b, c_, d = stat(7), stat(8), stat(9), stat(1)
            e, f_, g_ = stat(10), stat(11), stat(2)
            h, i_ = stat(12), stat(3)
            r0, r1, r2, r3 = stat(4), stat(5), stat(6), stat(0)

            # half 0 overlaps the tiles 4..7 stats work: lean on GPSIMD
            sched = itertools.cycle(
                [G, G, V] if hh == 0 else [V, V, G, V, G]
            )
            SH = [128, HT, NQ, 4]

            def tmp(name, pool=None, tag=None):
                pool = pool or lpool
                name = f"{name}_h{hh}"
                return pool.tile(SH, F32, tag=tag or name, name=name)

            def emul(u, v, name, pool=None, tag=None):
                t_ = tmp(name, pool, tag)
                next(sched).tensor_mul(out=t_, in0=u, in1=v)
                return t_

            def esub(u, v, name, pool=None, tag=None):
                t_ = tmp(name, pool, tag)
                next(sched).tensor_sub(out=t_, in0=u, in1=v)
                return t_

            def eadd(u, v, name, pool=None, tag=None):
                t_ = tmp(name, pool, tag)
                next(sched).tensor_add(out=t_, in0=u, in1=v)
                return t_

            def m2(u, v, w, x_, name):  # u*v - w*x
                p1 = emul(u, v, name + "p1", pppool, f"pp{hh}")
                p2 = emul(w, x_, name + "p2", pppool, f"pp{hh}")
                return esub(p1, p2, name)

            def asq(u, name):
                t_ = tmp(name)
                A.square(out=t_, in_=u)
                return t_

            def amul32(u, name):
                t_ = tmp(name)
                A.mul(out=t_, in_=u, mul=32.0)
                return t_

            b2 = asq(b, "b2")
            i2 = asq(i_, "i2")
            ae = emul(a, e, "aep1", pppool, f"pp{hh}")
            s01 = esub(ae, b2, "s01")
            s02 = m2(a, f_, c_, b, "s02")
            s03 = m2(a, g_, d, b, "s03")
            s12 = m2(b, f_, c_, e, "s12")
            s13 = m2(b, g_, d, e, "s13")
            s23 = m2(c_, g_, d, f_, "s23")
            s0R = m2(a, r1, r0, b, "s0R")
            s1R = m2(b, r1, r0, e, "s1R")
            s2R = m2(c_, r1, r0, f_, "s2R")
            s3R = m2(d, r1, r0, g_, "s3R")
            t01 = s23
            t02 = m2(c_, i_, h, d, "t02")
            t12 = m2(f_, i_, h, g_, "t12")
            c32 = amul32(c_, "c32")
            id_ = emul(i_, d, "idp1", pppool, f"pp{hh}")
            t03 = esub(c32, id_, "t03")
            fq32 = amul32(f_, "fq32")
            ig = emul(i_, g_, "igp1", pppool, f"pp{hh}")
            t13 = esub(fq32, ig, "t13")
            h32 = amul32(h, "h32")
            t23 = esub(h32, i2, "t23")
            t0R = m2(c_, r3, r2, d, "t0R")
            t1R = m2(f_, r3, r2, g_, "t1R")
            t2R = m2(h, r3, r2, i_, "t2R")
            ir3 = emul(i_, r3, "ir3p1", pppool, f"pp{hh}")
            r232 = amul32(r2, "r232")
            t3R = esub(ir3, r232, "t3R")

            Smap = {(0, 1): s01, (0, 2): s02, (0, 3): s03, (1, 2): s12,
                    (1, 3): s13, (2, 3): s23, (0, 4): s0R, (1, 4): s1R,
                    (2, 4): s2R, (3, 4): s3R}
            Tmap = {(0, 1): t01, (0, 2): t02, (0, 3): t03, (1, 2): t12,
                    (1, 3): t13, (2, 3): t23, (0, 4): t0R, (1, 4): t1R,
                    (2, 4): t2R, (3, 4): t3R}

            def det_terms(cols):
                terms = []
                for (p, q, sgn) in [(0, 1, 1.0), (0, 2, -1.0), (0, 3, 1.0),
                                    (1, 2, 1.0), (1, 3, -1.0), (2, 3, 1.0)]:
                    rest = [cols[k] for k in range(4) if k not in (p, q)]
                    cp, cq = cols[p], cols[q]
                    ss = 1.0
                    if cp > cq:
                        cp, cq, ss = cq, cp, -1.0
                    ta, tb = rest
                    ts = 1.0
                    if ta > tb:
                        ta, tb, ts = tb, ta, -1.0
                    terms.append((sgn * ss * ts, Smap[(cp, cq)],
                                  Tmap[(ta, tb)]))
                terms.sort(key=lambda z: -z[0])
                return terms

            def emit_det(cols, name):
                acc = None
                for idx, (sgn, sv, tv) in enumerate(det_terms(cols)):
                    if acc is None:
                        assert sgn > 0
                        acc = emul(sv, tv, f"{name}m{idx}", apool, f"acc{hh}")
                    else:
                        prod = emul(sv, tv, f"{name}m{idx}", pppool,
                                    f"pp{hh}")
                        if sgn > 0:
                            acc = eadd(acc, prod, f"{name}a{idx}", apool,
                                       f"acc{hh}")
                        else:
                            acc = esub(acc, prod, f"{name}a{idx}", apool,
                                       f"acc{hh}")
                return acc

            det = emit_det([0, 1, 2, 3], "dt")
            dR0 = emit_det([4, 1, 2, 3], "d0")
            dR1 = emit_det([0, 4, 2, 3], "d1")
            dR2 = emit_det([0, 1, 4, 3], "d2")
            dR3 = emit_det([0, 1, 2, 4], "d3")

            rdet = tmp("rdet")
            scratch = tmp("rscratch")
            V.reciprocal_approx_accurate(
                out=rdet.rearrange("p t q g -> p (t q g)"),
                in_=det.rearrange("p t q g -> p (t q g)"),
                scratch=scratch.rearrange("p t q g -> p (t q g)"),
            )
            nrdet = tmp("nrdet")
            V.tensor_scalar_mul(out=nrdet, in0=rdet, scalar1=-1.0)

            for comp, (dv, rv) in enumerate(
                [(dR0, nrdet), (dR1, nrdet), (dR2, nrdet), (dR3, rdet)]
            ):
                next(sched).tensor_mul(
                    out=OUT5[:, HT * hh:HT * (hh + 1), :, :, comp],
                    in0=dv, in1=rv,
                )


        for t in ([] if _skip_stats else range(NT)):
            xt = xpool.tile([128, CPT, M, D], F32, tag="xt", name="xt")
            nc.sync.dma_start(out=xt, in_=x_all[t])
            # cast + swizzle (c,m,d) -> (d, q, g, m) in one ACT pass
            xh = xhpool.tile([128, D, NQ, NG * M], F16, tag="xh", name="xh")
            A.copy(
                out=xh,
                in_=xt.rearrange("p (q g) m d -> p d q (g m)", q=NQ),
            )
            # PE transposes: blocks (d, q): [128, (g,m)] -> [(g,m), 128]
            fall = fpool.tile([128, D, NQ, 128], F16, tag="fall", name="fall")
            for dpair in range(2):
                pt = ptpool.tile([128, 2, NQ, 128], F16, tag="pt", name="pt")
                for dd in range(2):
                    d = dpair * 2 + dd
                    for q in range(NQ):
                        nc.tensor.transpose(
                            pt[:, dd, q, :], xh[:, d, q, :], ident16
                        )
                V.tensor_copy(
                    out=fall[:, dpair * 2:(dpair + 1) * 2, :, :], in_=pt
                )
            # fp16 products (feature-major, unit stride)
            prods = {}
            for ci, (s, j, k) in enumerate(CROSS):
                pr = prpool.tile([128, NQ, 128], F16, tag="pr", name="pr")
                eng = G if ci >= 4 else V
                eng.tensor_mul(out=pr, in0=fall[:, j], in1=fall[:, k])
                prods[s] = pr
            for sqi, (s, j) in enumerate(SQ):
                pr = prpool.tile([128, NQ, 128], F16, tag="pr", name="pr")
                if sqi == 0:
                    V.tensor_mul(out=pr, in0=fall[:, j], in1=fall[:, j])
                else:
                    A.square(out=pr, in_=fall[:, j])
                prods[s] = pr
            # PE reduce over m: 13 accumulating matmuls per 512-col window
            for w in range(2):
                spt = sppool.tile([52, 512], F32, tag="spt", name="spt")
                for s in range(NS):
                    if s < 4:
                        rhs = fall[:, s, 4 * w:4 * w + 4, :]
                    else:
                        rhs = prods[s][:, 4 * w:4 * w + 4, :]
                    nc.tensor.matmul(
                        spt,
                        MW[:, 48 - 4 * s:100 - 4 * s],
                        rhs,
                        start=(s == 0),
                        stop=(s == NS - 1),
                    )
                sst = sspool.tile([52, 512], F32, tag="sst", name="sst")
                A.copy(out=sst, in_=spt)
                # transpose stats back: chunks [52, 128] -> [128, 52]
                pt2 = p2pool.tile([128, 4, 52], F32, tag="pt2", name="pt2")
                for cidx in range(4):
                    nc.tensor.transpose(
                        pt2[:, cidx, :],
                        sst[:, cidx * 128:(cidx + 1) * 128],
                        ident32[0:52, 0:52],
                    )
                V.tensor_copy(
                    out=ST2h[t // HT][:, t % HT, 4 * w:4 * w + 4, :],
                    in_=pt2,
                )

        # ---------------- solve (identical math to strategy A) ----------
        if _skip_solve:
            OUTs = lpool.tile([128, NT, CPT, D], F32, tag="OUT", name="OUTs")
            V.tensor_copy(out=OUTs, in_=ST2h[0].rearrange(
                "p t q s -> p (t q s)")[:, 0:NT * CPT * D].rearrange(
                "p (t c d) -> p t c d", t=NT, c=CPT))
            nc.sync.dma_start(out=y_all, in_=OUTs)
            return

        emit_solve(0)
        emit_solve(1)
        nc.sync.dma_start(out=y_all, in_=OUT)


_NC_CACHE = {}


def _get_nc():
    if "nc" not in _NC_CACHE:
        nc = bacc.Bacc("TRN2", target_bir_lowering=False, debug=False,
                       num_devices=NCORES)
        xd = nc.dram_tensor("x", [BC, M, D], F32, kind="ExternalInput")
        yd = nc.dram_tensor("y", [BC, D], F32, kind="ExternalOutput")
        with tile.TileContext(nc) as tc:
            _emit(nc, tc, xd, yd)
        nc.compile()
        _NC_CACHE["nc"] = nc
    return _NC_CACHE["nc"]


def run_sharded(x, trace=False, **kwargs):
    nc = _get_nc()
    in_maps = [
        {"x": np.ascontiguousarray(x[k * BC:(k + 1) * BC])}
        for k in range(NCORES)
    ]
    res = run_bass_kernel_spmd(nc, in_maps, core_ids=list(range(NCORES)),
                               trace=trace, **kwargs)
    out = np.concatenate([res.results[k]["y"] for k in range(NCORES)], axis=0)
    return out, res


def kernel(**inputs):
    x = np.asarray(inputs["x"], dtype=np.float32)
    out, _ = run_sharded(x)
    return out
